# revision 28
# baseline (speedup 1.0000x reference)
"""Trainium2 Bass kernel for nn_EncoderTransformer_61194694033513.

Data-parallel over batch B=16 across 8 NeuronCores (2 batch elems per core).
Per core the forward runs out of SBUF, activations feature-major HT[e, tok]
in fp16, interleaved-chunk column layout: col = c*1024 + ec*512 + t'
(c = tok//512, t' = tok%512, ec = feat//128, partition = feat%128), so every
per-chunk op is one contiguous [P,1024] instruction covering both feature
chunks.

v3 structure:
- Wk folded into Wqk = Wq @ Wk^T on host; scores S = (H Wqk) H^T use live H
  tiles as keys; H double-buffered across layers (attention reads Hcur,
  writes Hcur+O into Hnxt).
- LN stats replicated across partitions via ones[P,128] lhsT matmuls so all
  row math runs 128-lane wide; no partition broadcasts.
- The dense stack (LN1+MLP+LN2+next-layer q'/v) is token-chunk-local, so it
  is embedded chunk-by-chunk inside the same stream's attention: attn(c0),
  attn(c1)||dense(c0), attn(c2)||dense(c1), attn(c3)||dense(c2), dense(c3).
  v is double-buffered across layers (next layer's v tiles are produced
  while this layer's attention still reads the old ones).
- The two batch elems additionally interleave half a layer out of phase.
- PSUM: s [P,512] ring2 (2 banks) + per-stream o [P,1024] ring1 (4 banks)
  + shared dense ring1 [P,1024] (2 banks) = 8 banks.
"""

import sys

import numpy as np

for _p in (
    "/opt/trn_rl_repo",
    "/opt/pypackages",
    "/root/.axon_site",
    "/root/.axon_site/_ro/trn_rl_repo",
    "/root/.axon_site/_ro/pypackages",
):
    if _p not in sys.path:
        sys.path.append(_p)

import concourse.bass as bass  # noqa: E402
import concourse.bacc as bacc  # noqa: E402
import concourse.mybir as mybir  # noqa: E402
from concourse import tile  # noqa: E402
from concourse.bass_utils import run_bass_kernel_spmd  # noqa: E402

B, N, D, E, L = 16, 2048, 128, 256, 3
NCORES = 8
BL = B // NCORES
P = 128
EC = E // P  # 2 feature chunks
CH = N // 512  # 4 token chunks
JT = N // P  # 16 key tiles
EPS = 1e-5
F32 = mybir.dt.float32
F16 = mybir.dt.float16
NPF16 = np.float16
AF = mybir.ActivationFunctionType
OP = mybir.AluOpType

_CACHE = {}


def _col(c, ec, off=0):
    return c * 1024 + ec * 512 + off


def _merge(xs, ys):
    """Proportionally interleave two task lists."""
    n, m = len(xs), len(ys)
    out = []
    i = j = 0
    while i < n or j < m:
        if j >= m or (i < n and i * (m + 1) <= j * (n + 1)):
            out.append(xs[i])
            i += 1
        else:
            out.append(ys[j])
            j += 1
    return out


def _build():
    nc = bacc.Bacc("TRN2", target_bir_lowering=False, debug=False, num_devices=NCORES)

    d_xsT = nc.declare_dram_parameter("xsT", [BL, P, N], F16, isOutput=False)
    d_Win = nc.declare_dram_parameter("Win", [D, E], F16, isOutput=False)
    WNAMES = ("Wqk", "Wv", "W1", "W2")
    d_W = {
        nm: nc.declare_dram_parameter(nm, [L, E, E], F16, isOutput=False)
        for nm in WNAMES
    }
    d_woutP = nc.declare_dram_parameter("woutP", [P, 2 * N], F16, isOutput=False)
    NCOL = 2 + 8 * L * EC
    d_colpack = nc.declare_dram_parameter("colpack", [P, NCOL], F32, isOutput=False)
    d_bout = nc.declare_dram_parameter("b_out", [1, 1], F32, isOutput=False)
    d_out = nc.declare_dram_parameter("out", [BL, 1], F32, isOutput=True)

    with tile.TileContext(nc) as tc:
        from contextlib import ExitStack

        with ExitStack() as ctx:
            cpool = ctx.enter_context(tc.tile_pool(name="const", bufs=1))
            hpool = ctx.enter_context(tc.tile_pool(name="acts", bufs=1))
            sqpool = ctx.enter_context(tc.tile_pool(name="sqp", bufs=2))
            spool = ctx.enter_context(tc.tile_pool(name="srelu", bufs=3))
            apool = ctx.enter_context(tc.tile_pool(name="mlpa", bufs=2))
            rpool = ctx.enter_context(tc.tile_pool(name="rowm", bufs=2))
            uppool = ctx.enter_context(tc.tile_pool(name="applyp", bufs=4))
            ropool = ctx.enter_context(tc.tile_pool(name="ro", bufs=2))

            PS = bass.MemorySpace.PSUM
            # s: 2x[P,1024] (4 banks), o: 1x[P,1024] (2), d: 1x[P,1024] (2)
            ps_s = ctx.enter_context(tc.tile_pool(name="ps_s", bufs=2, space=PS))
            ps_o = ctx.enter_context(tc.tile_pool(name="ps_o", bufs=1, space=PS))
            ps_d = ctx.enter_context(tc.tile_pool(name="ps_d", bufs=1, space=PS))

            xs_tiles = []
            for b in range(BL):
                xt = hpool.tile([P, N], F16, name=f"xst{b}", tag=f"xst{b}")
                for c in range(CH):
                    cs = slice(c * 512, (c + 1) * 512)
                    nc.sync.dma_start(xt[:, cs], d_xsT[b][:, cs])
                xs_tiles.append(xt)

            win_sb = cpool.tile([P, E], F16, name="win", tag="win")
            nc.sync.dma_start(win_sb[:], d_Win[:])
            colpack = cpool.tile([P, NCOL], F32, name="colpack", tag="colpack")
            nc.sync.dma_start(colpack[:], d_colpack[:])
            binp_sb = colpack[:, 0:EC]

            def col_views(base):
                return [
                    colpack[
                        :, 2 + base * L * EC + l * EC : 2 + base * L * EC + (l + 1) * EC
                    ]
                    for l in range(L)
                ]

            bm1_sb = col_views(0)
            bm2_sb = col_views(1)
            be1_sb = col_views(2)
            be2_sb = col_views(3)
            g1_sb = col_views(4)
            g2_sb = col_views(5)
            ng1_sb = col_views(6)
            ng2_sb = col_views(7)

            w_sb = {nm: [None] * L for nm in WNAMES}
            w_big = {}
            for nm in WNAMES:
                w_big[nm] = [
                    cpool.tile([P, L * E], F16, name=f"{nm}B{ec}", tag=f"{nm}B{ec}")
                    for ec in range(EC)
                ]
            for l in range(L):
                for nm in WNAMES:
                    for ec in range(EC):
                        nc.sync.dma_start(
                            w_big[nm][ec][:, l * E : (l + 1) * E],
                            d_W[nm][l, ec * P : (ec + 1) * P, :],
                        )
            for nm in WNAMES:
                for l in range(L):
                    w_sb[nm][l] = [
                        w_big[nm][ec][:, l * E : (l + 1) * E] for ec in range(EC)
                    ]

            wout_sb = cpool.tile([P, 2 * N], F16, name="woutp", tag="woutp")
            nc.sync.dma_start(wout_sb[:], d_woutP[:])
            bout_sb = cpool.tile([1, 1], F32, name="bout", tag="bout")
            nc.sync.dma_start(bout_sb[:], d_bout[:])

            ones128 = cpool.tile([P, P], F16, name="ones128", tag="ones128")
            nc.vector.memset(ones128[:], 1.0)
            epsc = cpool.tile([P, 1], F32, name="epsc", tag="epsc")
            nc.vector.memset(epsc[:], EPS)

            Hbuf = [
                [
                    hpool.tile([P, 2 * N], F16, name=f"H{b}{i}", tag=f"H{b}{i}")
                    for i in range(2)
                ]
                for b in range(BL)
            ]
            qT = [
                hpool.tile([P, 2 * N], F16, name=f"qT{b}", tag=f"qT{b}")
                for b in range(BL)
            ]
            vbuf = [
                [
                    hpool.tile([P, JT * E], F16, name=f"v{b}{i}", tag=f"v{b}{i}")
                    for i in range(2)
                ]
                for b in range(BL)
            ]
            rstd_all = [
                hpool.tile([P, N], F16, name=f"rstd{b}", tag=f"rstd{b}")
                for b in range(BL)
            ]
            mrstd_all = [
                hpool.tile([P, N], F16, name=f"mrstd{b}", tag=f"mrstd{b}")
                for b in range(BL)
            ]
            rsum4 = [
                hpool.tile([P, CH], F32, name=f"rs4{b}", tag=f"rs4{b}")
                for b in range(BL)
            ]

            relu_ctr = [0, 0]
            sq_ctr = [0, 0]

            def proj_tasks(b):
                def mk(c):
                    def t():
                        ps = ps_d.tile([P, 1024], F32, name="psp", tag="d")
                        for ec in range(EC):
                            nc.tensor.matmul(
                                ps[:, ec * 512 : (ec + 1) * 512],
                                win_sb[:, ec * P : (ec + 1) * P],
                                xs_tiles[b][:, c * 512 : (c + 1) * 512],
                            )
                        for ec in range(EC):
                            nc.scalar.activation(
                                Hbuf[b][0][:, _col(c, ec) : _col(c, ec) + 512],
                                ps[:, ec * 512 : (ec + 1) * 512],
                                AF.Identity,
                                bias=binp_sb[:, ec : ec + 1],
                            )

                    return t

                return [mk(c) for c in range(CH)]

            def qv_chunk_tasks(b, l, c):
                """q' chunk c and v key tiles 4c..4c+3 for layer l (reads
                H_l = Hbuf[b][l%2], writes qT and vbuf[b][l%2])."""
                cur = Hbuf[b][l % 2]
                vd = vbuf[b][l % 2]
                tasks = []

                def tq():
                    ps = ps_d.tile([P, 1024], F32, name="psq", tag="d")
                    for dc in range(EC):
                        for ec in range(EC):
                            nc.tensor.matmul(
                                ps[:, dc * 512 : (dc + 1) * 512],
                                w_sb["Wqk"][l][ec][:, dc * P : (dc + 1) * P],
                                cur[:, _col(c, ec) : _col(c, ec) + 512],
                                start=(ec == 0),
                                stop=(ec == EC - 1),
                            )
                    nc.scalar.copy(qT[b][:, c * 1024 : (c + 1) * 1024], ps[:])

                tasks.append(tq)

                def mk_v(t2):
                    def t():
                        ps = ps_d.tile([P, 1024], F32, name="psv", tag="d")
                        for h in range(2):
                            kt = 2 * t2 + h
                            for ec in range(EC):
                                nc.tensor.matmul(
                                    ps[:, h * E : (h + 1) * E],
                                    cur[
                                        :,
                                        _col(kt // 4, ec, (kt % 4) * P) : _col(
                                            kt // 4, ec, (kt % 4) * P
                                        )
                                        + P,
                                    ],
                                    w_sb["Wv"][l][ec][:],
                                    start=(ec == 0),
                                    stop=(ec == EC - 1),
                                )
                        dst = vd[:, 2 * t2 * E : (2 * t2 + 2) * E]
                        nc.scalar.copy(dst, ps[:, 0 : 2 * E])

                    return t

                tasks.append(mk_v(2 * c))
                tasks.append(mk_v(2 * c + 1))
                return tasks

            def attn_chunk_tasks(b, l, c, state):
                cur = Hbuf[b][l % 2]
                nxt = Hbuf[b][(l + 1) % 2]
                vd = vbuf[b][l % 2]
                tasks = []

                def mk_j2(j2):
                    def t():
                        if j2 == 0:
                            state["o"] = ps_o.tile([P, 1024], F32, name="o", tag="o")
                        o_ps = state["o"]
                        s_ps = ps_s.tile([P, 1024], F32, name="s", tag="s")
                        for h in range(2):
                            j = 2 * j2 + h
                            for dc in range(EC):
                                nc.tensor.matmul(
                                    s_ps[:, h * 512 : (h + 1) * 512],
                                    cur[
                                        :,
                                        _col(j // 4, dc, (j % 4) * P) : _col(
                                            j // 4, dc, (j % 4) * P
                                        )
                                        + P,
                                    ],
                                    qT[b][:, _col(c, dc) : _col(c, dc) + 512],
                                    start=(dc == 0),
                                    stop=(dc == EC - 1),
                                )
                        sr = spool.tile([P, 1024], F16, name="sr", tag="sr")
                        if relu_ctr[b] % 8 < 3:
                            nc.vector.tensor_relu(sr[:], s_ps[:])
                        else:
                            nc.scalar.activation(sr[:], s_ps[:], AF.Relu)
                        relu_ctr[b] += 1
                        for h in range(2):
                            j = 2 * j2 + h
                            for oc in range(EC):
                                nc.tensor.matmul(
                                    o_ps[:, oc * 512 : (oc + 1) * 512],
                                    vd[:, j * E + oc * P : j * E + (oc + 1) * P],
                                    sr[:, h * 512 : (h + 1) * 512],
                                    start=(j == 0),
                                    stop=(j == JT - 1),
                                )

                    return t

                for j2 in range(JT // 2):
                    tasks.append(mk_j2(j2))

                def t_add():
                    nc.vector.tensor_add(
                        nxt[:, c * 1024 : (c + 1) * 1024],
                        cur[:, c * 1024 : (c + 1) * 1024],
                        state["o"][:],
                    )

                tasks.append(t_add)
                return tasks

            def ln_chunk_tasks(b, X, c, g_col, ng_col, be_col):
                def t_stats():
                    sq = sqpool.tile([P, 1024], F16, name="sq", tag="sq")
                    # sq = (X/64)^2: pre-LN |X| can exceed 2000, so X^2
                    # overflows fp16 without the pre-scale
                    nc.scalar.activation(
                        sq[:],
                        X[:, c * 1024 : (c + 1) * 1024],
                        AF.Square,
                        scale=1.0 / 64,
                    )
                    st = ps_d.tile([P, 1024], F32, name="st", tag="d")
                    st_s = st[:, 0:512]
                    st_q = st[:, 512:1024]
                    for ec in range(EC):
                        nc.tensor.matmul(
                            st_s,
                            ones128[:],
                            X[:, _col(c, ec) : _col(c, ec) + 512],
                            start=(ec == 0),
                            stop=(ec == EC - 1),
                        )
                    for ec in range(EC):
                        nc.tensor.matmul(
                            st_q,
                            ones128[:],
                            sq[:, ec * 512 : (ec + 1) * 512],
                            start=(ec == 0),
                            stop=(ec == EC - 1),
                        )
                    cs = slice(c * 512, (c + 1) * 512)
                    t1 = rpool.tile([P, 512], F32, name="t1", tag="t1")
                    # t1 = (sum/64)^2 = sum^2/4096 (one PSUM input only)
                    nc.scalar.activation(t1[:], st_s, AF.Square, scale=1.0 / 64)
                    # t1 <- E*var/4096 = (sumsq - sum^2/E)/4096
                    nc.vector.scalar_tensor_tensor(
                        t1[:], t1[:], -1.0 / E, st_q, op0=OP.mult, op1=OP.add
                    )
                    nc.scalar.activation(
                        rstd_all[b][:, cs],
                        t1[:],
                        AF.Abs_reciprocal_sqrt,
                        bias=epsc[:],
                        scale=4096.0 / E,
                    )
                    nc.vector.scalar_tensor_tensor(
                        mrstd_all[b][:, cs],
                        st_s,
                        1.0 / E,
                        rstd_all[b][:, cs],
                        op0=OP.mult,
                        op1=OP.mult,
                    )

                def t_apply():
                    cs = slice(c * 512, (c + 1) * 512)
                    up = uppool.tile([P, 1024], F16, name="up", tag="up")
                    tt = uppool.tile([P, 1024], F16, name="tt", tag="tt")
                    # alternate GP/DVE per chunk so neither engine serializes
                    # the apply stage (GP runs ~2.3x slower per element)
                    up_eng = nc.gpsimd if c % 2 == 0 else nc.vector
                    add_eng = nc.gpsimd if c % 2 == 1 else nc.vector
                    for pt in range(EC):
                        up_eng.tensor_scalar(
                            up[:, pt * 512 : (pt + 1) * 512],
                            mrstd_all[b][:, cs],
                            ng_col[:, pt : pt + 1],
                            be_col[:, pt : pt + 1],
                            op0=OP.mult,
                            op1=OP.add,
                        )
                        nc.vector.scalar_tensor_tensor(
                            tt[:, pt * 512 : (pt + 1) * 512],
                            X[:, _col(c, pt) : _col(c, pt) + 512],
                            g_col[:, pt : pt + 1],
                            rstd_all[b][:, cs],
                            op0=OP.mult,
                            op1=OP.mult,
                        )
                    add_eng.tensor_add(
                        X[:, c * 1024 : (c + 1) * 1024], tt[:], up[:]
                    )

                return [t_stats, t_apply]

            def mlp_chunk_task(b, l, c):
                X = Hbuf[b][(l + 1) % 2]

                def t():
                    psa = ps_d.tile([P, 1024], F32, name="psa", tag="d")
                    for mc in range(EC):
                        for ec in range(EC):
                            nc.tensor.matmul(
                                psa[:, mc * 512 : (mc + 1) * 512],
                                w_sb["W1"][l][ec][:, mc * P : (mc + 1) * P],
                                X[:, _col(c, ec) : _col(c, ec) + 512],
                                start=(ec == 0),
                                stop=(ec == EC - 1),
                            )
                    a = apool.tile([P, 1024], F16, name="a", tag="a")
                    for mc in range(EC):
                        nc.scalar.activation(
                            a[:, mc * 512 : (mc + 1) * 512],
                            psa[:, mc * 512 : (mc + 1) * 512],
                            AF.Relu,
                            bias=bm1_sb[l][:, mc : mc + 1],
                        )
                    psm = ps_d.tile([P, 1024], F32, name="psm", tag="d")
                    for oc in range(EC):
                        for mc in range(EC):
                            nc.tensor.matmul(
                                psm[:, oc * 512 : (oc + 1) * 512],
                                w_sb["W2"][l][mc][:, oc * P : (oc + 1) * P],
                                a[:, mc * 512 : (mc + 1) * 512],
                                start=(mc == 0),
                                stop=(mc == EC - 1),
                            )
                    for oc in range(EC):
                        xs_ = slice(_col(c, oc), _col(c, oc) + 512)
                        nc.vector.scalar_tensor_tensor(
                            X[:, xs_],
                            psm[:, oc * 512 : (oc + 1) * 512],
                            bm2_sb[l][:, oc : oc + 1],
                            X[:, xs_],
                            op0=OP.add,
                            op1=OP.add,
                        )

                return [t]

            def ro_chunk_tasks(b, c):
                X = Hbuf[b][L % 2]

                def t():
                    ros = ropool.tile([P, 1024], F16, name="ros", tag="ros")
                    nc.vector.tensor_mul(
                        ros[:],
                        X[:, c * 1024 : (c + 1) * 1024],
                        wout_sb[:, c * 1024 : (c + 1) * 1024],
                    )
                    st = ps_d.tile([P, 1024], F32, name="str", tag="d")
                    for ec in range(EC):
                        nc.tensor.matmul(
                            st[:, 0:512],
                            ones128[:],
                            ros[:, ec * 512 : (ec + 1) * 512],
                            start=(ec == 0),
                            stop=(ec == EC - 1),
                        )
                    nc.vector.reduce_sum(
                        rsum4[b][:, c : c + 1], st[:, 0:512], axis=mybir.AxisListType.X
                    )

                return [t]

            def ro_final_task(b):
                def t():
                    rsc = ropool.tile([P, 1], F32, name="rsc", tag="rsc")
                    nc.vector.reduce_sum(
                        rsc[:], rsum4[b][:], axis=mybir.AxisListType.X
                    )
                    ob = ropool.tile([1, 1], F32, name="ob", tag="ob")
                    nc.scalar.activation(
                        ob[:], rsc[0:1, :], AF.Identity, bias=bout_sb[:]
                    )
                    nc.sync.dma_start(d_out[b : b + 1, :], ob[:])

                return [t]

            def dense_tasks(b, l):
                """Stage-major dense phase for layer l: each stage sweeps all
                chunks so per-chunk serial chains (stats->rowmath->apply)
                pipeline across chunks instead of stalling the next stage."""
                X = Hbuf[b][(l + 1) % 2]
                S = []
                ln1 = [
                    ln_chunk_tasks(b, X, c, g1_sb[l], ng1_sb[l], be1_sb[l])
                    for c in range(CH)
                ]
                ln2 = [
                    ln_chunk_tasks(b, X, c, g2_sb[l], ng2_sb[l], be2_sb[l])
                    for c in range(CH)
                ]
                S += [t[0] for t in ln1]  # stats c0..c3
                S += [t[1] for t in ln1]  # applies c0..c3
                for c in range(CH):
                    S += mlp_chunk_task(b, l, c)
                S += [t[0] for t in ln2]
                S += [t[1] for t in ln2]
                for c in range(CH):
                    if l + 1 < L:
                        S += qv_chunk_tasks(b, l + 1, c)
                    else:
                        S += ro_chunk_tasks(b, c)
                return S

            def attn_tasks(b, l):
                state = {}
                S = []
                for c in range(CH):
                    S += attn_chunk_tasks(b, l, c, state)
                return S

            def qv0_tasks(b):
                S = []
                for c in range(CH):
                    S += qv_chunk_tasks(b, 0, c)
                return S

            # Phase-disjoint pairing: one stream's attention (PE-heavy)
            # always overlaps the other stream's dense stack (element-engine
            # heavy). Attention phases of the two streams never overlap, so
            # the shared s/o PSUM rings hand off only at phase boundaries
            # (all WAR edges point backwards in emission order: no cycles).
            A, Bb = 0, 1
            sched = []
            sched += _merge(proj_tasks(A), proj_tasks(Bb))
            sched += _merge(qv0_tasks(A) + attn_tasks(A, 0), qv0_tasks(Bb))
            sched += _merge(dense_tasks(A, 0), attn_tasks(Bb, 0))
            sched += _merge(attn_tasks(A, 1), dense_tasks(Bb, 0))
            sched += _merge(dense_tasks(A, 1), attn_tasks(Bb, 1))
            sched += _merge(attn_tasks(A, 2), dense_tasks(Bb, 1))
            # Tail: B's last layer self-embeds its dense stack chunk-by-chunk
            # inside its own attention (chunk c's dense only needs chunk c's
            # H-add), so only dense(c3)+ro remain exposed at the very end.
            stateB = {}

            def denseB2(c):
                X = Hbuf[Bb][L % 2]
                ln1 = ln_chunk_tasks(Bb, X, c, g1_sb[2], ng1_sb[2], be1_sb[2])
                ln2 = ln_chunk_tasks(Bb, X, c, g2_sb[2], ng2_sb[2], be2_sb[2])
                return (
                    [ln1[0], ln1[1]]
                    + mlp_chunk_task(Bb, 2, c)
                    + [ln2[0], ln2[1]]
                    + ro_chunk_tasks(Bb, c)
                )

            tailB = attn_chunk_tasks(Bb, 2, 0, stateB)
            for c in range(1, CH):
                tailB += _merge(attn_chunk_tasks(Bb, 2, c, stateB), denseB2(c - 1))
            tailB += denseB2(CH - 1)
            sched += _merge(dense_tasks(A, 2) + ro_final_task(A), tailB)
            sched += ro_final_task(Bb)
            for t in sched:
                t()

    nc.compile()
    return nc


def _prep_inputs(inputs):
    f = lambda x: np.asarray(x, np.float32)
    xs = f(inputs["xs"])
    xsT = np.ascontiguousarray(xs.transpose(0, 2, 1)).astype(NPF16)  # [B, D, N]
    Wq, Wk = f(inputs["Wq"]), f(inputs["Wk"])
    Wqk = np.stack([Wq[l] @ Wk[l].T for l in range(L)]).astype(NPF16)
    WoutT = f(inputs["Wout"]).reshape(N, E).T  # [E, N]
    woutP = np.ascontiguousarray(
        WoutT.reshape(EC, P, CH, 512).transpose(1, 2, 0, 3).reshape(P, 2 * N)
    ).astype(NPF16)

    def cols(v, per_l):
        v = f(v)
        if per_l:
            return np.ascontiguousarray(v.reshape(L, EC, P).transpose(0, 2, 1))
        return np.ascontiguousarray(v.reshape(EC, P).T)

    groups = [
        cols(inputs["bm1"], True),
        cols(inputs["bm2"], True),
        cols(inputs["be1"], True),
        cols(inputs["be2"], True),
        cols(inputs["g1"], True),
        cols(inputs["g2"], True),
        cols(-f(inputs["g1"]), True),
        cols(-f(inputs["g2"]), True),
    ]
    colpack = np.concatenate(
        [cols(inputs["b_in"], False)]
        + [g.transpose(1, 0, 2).reshape(P, L * EC) for g in groups],
        axis=1,
    )
    common = {
        "Win": f(inputs["Win"]).astype(NPF16),
        "Wqk": Wqk,
        "Wv": f(inputs["Wv"]).astype(NPF16),
        "W1": f(inputs["W1"]).astype(NPF16),
        "W2": f(inputs["W2"]).astype(NPF16),
        "woutP": woutP,
        "colpack": np.ascontiguousarray(colpack),
        "b_out": f(inputs["b_out"]).reshape(1, 1),
    }
    in_maps = []
    for c in range(NCORES):
        m = dict(common)
        m["xsT"] = np.ascontiguousarray(xsT[c * BL : (c + 1) * BL])
        in_maps.append(m)
    return in_maps


def get_program():
    if "nc" not in _CACHE:
        _CACHE["nc"] = _build()
    return _CACHE["nc"]


def kernel(**inputs) -> np.ndarray:
    nc = get_program()
    in_maps = _prep_inputs(inputs)
    res = run_bass_kernel_spmd(nc, in_maps, list(range(NCORES)))
    out = np.concatenate([res.results[c]["out"] for c in range(NCORES)], axis=0)
    return out.astype(np.float32)


# revision 31
# speedup vs baseline: 1.0296x; 1.0296x over previous
"""Trainium2 Bass kernel for nn_EncoderTransformer_61194694033513.

Data-parallel over batch B=16 across 8 NeuronCores (2 batch elems per core).
Per core the forward runs out of SBUF, activations feature-major HT[e, tok]
in fp16, interleaved-chunk column layout: col = c*1024 + ec*512 + t'
(c = tok//512, t' = tok%512, ec = feat//128, partition = feat%128), so every
per-chunk op is one contiguous [P,1024] instruction covering both feature
chunks.

v3 structure:
- Wk folded into Wqk = Wq @ Wk^T on host; scores S = (H Wqk) H^T use live H
  tiles as keys; H double-buffered across layers (attention reads Hcur,
  writes Hcur+O into Hnxt).
- LN stats replicated across partitions via ones[P,128] lhsT matmuls so all
  row math runs 128-lane wide; no partition broadcasts.
- The dense stack (LN1+MLP+LN2+next-layer q'/v) is token-chunk-local, so it
  is embedded chunk-by-chunk inside the same stream's attention: attn(c0),
  attn(c1)||dense(c0), attn(c2)||dense(c1), attn(c3)||dense(c2), dense(c3).
  v is double-buffered across layers (next layer's v tiles are produced
  while this layer's attention still reads the old ones).
- The two batch elems additionally interleave half a layer out of phase.
- PSUM: s [P,512] ring2 (2 banks) + per-stream o [P,1024] ring1 (4 banks)
  + shared dense ring1 [P,1024] (2 banks) = 8 banks.
"""

import sys

import numpy as np

for _p in (
    "/opt/trn_rl_repo",
    "/opt/pypackages",
    "/root/.axon_site",
    "/root/.axon_site/_ro/trn_rl_repo",
    "/root/.axon_site/_ro/pypackages",
):
    if _p not in sys.path:
        sys.path.append(_p)

import concourse.bass as bass  # noqa: E402
import concourse.bacc as bacc  # noqa: E402
import concourse.mybir as mybir  # noqa: E402
from concourse import tile  # noqa: E402
from concourse.bass_utils import run_bass_kernel_spmd  # noqa: E402

B, N, D, E, L = 16, 2048, 128, 256, 3
NCORES = 8
BL = B // NCORES
P = 128
EC = E // P  # 2 feature chunks
CH = N // 512  # 4 token chunks
JT = N // P  # 16 key tiles
EPS = 1e-5
F32 = mybir.dt.float32
F16 = mybir.dt.float16
NPF16 = np.float16
AF = mybir.ActivationFunctionType
OP = mybir.AluOpType

_CACHE = {}


def _col(c, ec, off=0):
    return c * 1024 + ec * 512 + off


def _merge(xs, ys):
    """Proportionally interleave two task lists."""
    n, m = len(xs), len(ys)
    out = []
    i = j = 0
    while i < n or j < m:
        if j >= m or (i < n and i * (m + 1) <= j * (n + 1)):
            out.append(xs[i])
            i += 1
        else:
            out.append(ys[j])
            j += 1
    return out


def _build():
    nc = bacc.Bacc("TRN2", target_bir_lowering=False, debug=False, num_devices=NCORES)

    d_xsT = nc.declare_dram_parameter("xsT", [BL, P, N], F16, isOutput=False)
    d_Win = nc.declare_dram_parameter("Win", [D, E], F16, isOutput=False)
    WNAMES = ("Wqk", "Wv", "W1", "W2")
    d_W = {
        nm: nc.declare_dram_parameter(nm, [L, E, E], F16, isOutput=False)
        for nm in WNAMES
    }
    d_woutP = nc.declare_dram_parameter("woutP", [P, 2 * N], F16, isOutput=False)
    NCOL = 2 + 8 * L * EC
    d_colpack = nc.declare_dram_parameter("colpack", [P, NCOL], F32, isOutput=False)
    d_bout = nc.declare_dram_parameter("b_out", [1, 1], F32, isOutput=False)
    d_out = nc.declare_dram_parameter("out", [BL, 1], F32, isOutput=True)

    with tile.TileContext(nc) as tc:
        from contextlib import ExitStack

        with ExitStack() as ctx:
            cpool = ctx.enter_context(tc.tile_pool(name="const", bufs=1))
            hpool = ctx.enter_context(tc.tile_pool(name="acts", bufs=1))
            sqpool = ctx.enter_context(tc.tile_pool(name="sqp", bufs=2))
            spool = ctx.enter_context(tc.tile_pool(name="srelu", bufs=3))
            apool = ctx.enter_context(tc.tile_pool(name="mlpa", bufs=2))
            rpool = ctx.enter_context(tc.tile_pool(name="rowm", bufs=2))
            uppool = ctx.enter_context(tc.tile_pool(name="applyp", bufs=4))
            ropool = ctx.enter_context(tc.tile_pool(name="ro", bufs=2))

            PS = bass.MemorySpace.PSUM
            # s: 2x[P,1024] (4 banks), o: 1x[P,1024] (2), d: 1x[P,1024] (2)
            ps_s = ctx.enter_context(tc.tile_pool(name="ps_s", bufs=2, space=PS))
            ps_o = ctx.enter_context(tc.tile_pool(name="ps_o", bufs=1, space=PS))
            ps_d = ctx.enter_context(tc.tile_pool(name="ps_d", bufs=1, space=PS))

            xs_tiles = []
            for b in range(BL):
                xt = hpool.tile([P, N], F16, name=f"xst{b}", tag=f"xst{b}")
                for c in range(CH):
                    cs = slice(c * 512, (c + 1) * 512)
                    nc.sync.dma_start(xt[:, cs], d_xsT[b][:, cs])
                xs_tiles.append(xt)

            win_sb = cpool.tile([P, E], F16, name="win", tag="win")
            nc.sync.dma_start(win_sb[:], d_Win[:])
            colpack = cpool.tile([P, NCOL], F32, name="colpack", tag="colpack")
            nc.sync.dma_start(colpack[:], d_colpack[:])
            binp_sb = colpack[:, 0:EC]

            def col_views(base):
                return [
                    colpack[
                        :, 2 + base * L * EC + l * EC : 2 + base * L * EC + (l + 1) * EC
                    ]
                    for l in range(L)
                ]

            bm1_sb = col_views(0)
            bm2_sb = col_views(1)
            be1_sb = col_views(2)
            be2_sb = col_views(3)
            g1_sb = col_views(4)
            g2_sb = col_views(5)
            ng1_sb = col_views(6)
            ng2_sb = col_views(7)

            w_sb = {nm: [None] * L for nm in WNAMES}
            w_big = {}
            for nm in WNAMES:
                w_big[nm] = [
                    cpool.tile([P, L * E], F16, name=f"{nm}B{ec}", tag=f"{nm}B{ec}")
                    for ec in range(EC)
                ]
            for l in range(L):
                for nm in WNAMES:
                    for ec in range(EC):
                        nc.sync.dma_start(
                            w_big[nm][ec][:, l * E : (l + 1) * E],
                            d_W[nm][l, ec * P : (ec + 1) * P, :],
                        )
            for nm in WNAMES:
                for l in range(L):
                    w_sb[nm][l] = [
                        w_big[nm][ec][:, l * E : (l + 1) * E] for ec in range(EC)
                    ]

            wout_sb = cpool.tile([P, 2 * N], F16, name="woutp", tag="woutp")
            nc.sync.dma_start(wout_sb[:], d_woutP[:])
            bout_sb = cpool.tile([1, 1], F32, name="bout", tag="bout")
            nc.sync.dma_start(bout_sb[:], d_bout[:])

            ones128 = cpool.tile([P, P], F16, name="ones128", tag="ones128")
            nc.vector.memset(ones128[:], 1.0)
            epsc = cpool.tile([P, 1], F32, name="epsc", tag="epsc")
            nc.vector.memset(epsc[:], EPS)
            # pre-warm ACT function tables during the initial DMA wait so the
            # first real activation doesn't pay ACT_TABLE_LOAD
            warm = cpool.tile([1, 1], F32, name="warm", tag="warm")
            nc.vector.memset(warm[:], 1.0)
            for fn in (AF.Relu, AF.Square, AF.Abs_reciprocal_sqrt, AF.Identity):
                nc.scalar.activation(warm[:], warm[:], fn)

            Hbuf = [
                [
                    hpool.tile([P, 2 * N], F16, name=f"H{b}{i}", tag=f"H{b}{i}")
                    for i in range(2)
                ]
                for b in range(BL)
            ]
            qT = [
                hpool.tile([P, 2 * N], F16, name=f"qT{b}", tag=f"qT{b}")
                for b in range(BL)
            ]
            vbuf = [
                [
                    hpool.tile([P, JT * E], F16, name=f"v{b}{i}", tag=f"v{b}{i}")
                    for i in range(2)
                ]
                for b in range(BL)
            ]
            rstd_all = [
                hpool.tile([P, N], F16, name=f"rstd{b}", tag=f"rstd{b}")
                for b in range(BL)
            ]
            mrstd_all = [
                hpool.tile([P, N], F16, name=f"mrstd{b}", tag=f"mrstd{b}")
                for b in range(BL)
            ]
            rsum4 = [
                hpool.tile([P, CH], F32, name=f"rs4{b}", tag=f"rs4{b}")
                for b in range(BL)
            ]

            relu_ctr = [0, 0]
            sq_ctr = [0, 0]

            def proj_tasks(b):
                def mk(c):
                    def t():
                        # ps_s ring (idle pre-attention): deeper pipelining
                        # than the shared dense ring during startup
                        ps = ps_s.tile([P, 1024], F32, name="psp", tag="s")
                        for ec in range(EC):
                            nc.tensor.matmul(
                                ps[:, ec * 512 : (ec + 1) * 512],
                                win_sb[:, ec * P : (ec + 1) * P],
                                xs_tiles[b][:, c * 512 : (c + 1) * 512],
                            )
                        for ec in range(EC):
                            nc.scalar.activation(
                                Hbuf[b][0][:, _col(c, ec) : _col(c, ec) + 512],
                                ps[:, ec * 512 : (ec + 1) * 512],
                                AF.Identity,
                                bias=binp_sb[:, ec : ec + 1],
                            )

                    return t

                return [mk(c) for c in range(CH)]

            def qv_chunk_tasks(b, l, c):
                """q' chunk c and v key tiles 4c..4c+3 for layer l (reads
                H_l = Hbuf[b][l%2], writes qT and vbuf[b][l%2])."""
                cur = Hbuf[b][l % 2]
                vd = vbuf[b][l % 2]
                tasks = []

                def tq():
                    ps = ps_d.tile([P, 1024], F32, name="psq", tag="d")
                    for dc in range(EC):
                        for ec in range(EC):
                            nc.tensor.matmul(
                                ps[:, dc * 512 : (dc + 1) * 512],
                                w_sb["Wqk"][l][ec][:, dc * P : (dc + 1) * P],
                                cur[:, _col(c, ec) : _col(c, ec) + 512],
                                start=(ec == 0),
                                stop=(ec == EC - 1),
                            )
                    nc.scalar.copy(qT[b][:, c * 1024 : (c + 1) * 1024], ps[:])

                tasks.append(tq)

                def mk_v(t2):
                    def t():
                        ps = ps_d.tile([P, 1024], F32, name="psv", tag="d")
                        for h in range(2):
                            kt = 2 * t2 + h
                            for ec in range(EC):
                                nc.tensor.matmul(
                                    ps[:, h * E : (h + 1) * E],
                                    cur[
                                        :,
                                        _col(kt // 4, ec, (kt % 4) * P) : _col(
                                            kt // 4, ec, (kt % 4) * P
                                        )
                                        + P,
                                    ],
                                    w_sb["Wv"][l][ec][:],
                                    start=(ec == 0),
                                    stop=(ec == EC - 1),
                                )
                        dst = vd[:, 2 * t2 * E : (2 * t2 + 2) * E]
                        nc.scalar.copy(dst, ps[:, 0 : 2 * E])

                    return t

                tasks.append(mk_v(2 * c))
                tasks.append(mk_v(2 * c + 1))
                return tasks

            def attn_chunk_tasks(b, l, c, state):
                cur = Hbuf[b][l % 2]
                nxt = Hbuf[b][(l + 1) % 2]
                vd = vbuf[b][l % 2]
                tasks = []

                def mk_j2(j2):
                    def t():
                        if j2 == 0:
                            state["o"] = ps_o.tile([P, 1024], F32, name="o", tag="o")
                        o_ps = state["o"]
                        s_ps = ps_s.tile([P, 1024], F32, name="s", tag="s")
                        for h in range(2):
                            j = 2 * j2 + h
                            for dc in range(EC):
                                nc.tensor.matmul(
                                    s_ps[:, h * 512 : (h + 1) * 512],
                                    cur[
                                        :,
                                        _col(j // 4, dc, (j % 4) * P) : _col(
                                            j // 4, dc, (j % 4) * P
                                        )
                                        + P,
                                    ],
                                    qT[b][:, _col(c, dc) : _col(c, dc) + 512],
                                    start=(dc == 0),
                                    stop=(dc == EC - 1),
                                )
                        sr = spool.tile([P, 1024], F16, name="sr", tag="sr")
                        if relu_ctr[b] % 8 < 3:
                            nc.vector.tensor_relu(sr[:], s_ps[:])
                        else:
                            nc.scalar.activation(sr[:], s_ps[:], AF.Relu)
                        relu_ctr[b] += 1
                        for h in range(2):
                            j = 2 * j2 + h
                            for oc in range(EC):
                                nc.tensor.matmul(
                                    o_ps[:, oc * 512 : (oc + 1) * 512],
                                    vd[:, j * E + oc * P : j * E + (oc + 1) * P],
                                    sr[:, h * 512 : (h + 1) * 512],
                                    start=(j == 0),
                                    stop=(j == JT - 1),
                                )

                    return t

                for j2 in range(JT // 2):
                    tasks.append(mk_j2(j2))

                def t_add():
                    nc.vector.tensor_add(
                        nxt[:, c * 1024 : (c + 1) * 1024],
                        cur[:, c * 1024 : (c + 1) * 1024],
                        state["o"][:],
                    )

                tasks.append(t_add)
                return tasks

            def ln_chunk_tasks(b, X, c, g_col, ng_col, be_col):
                def t_stats():
                    sq = sqpool.tile([P, 1024], F16, name="sq", tag="sq")
                    # sq = (X/64)^2: pre-LN |X| can exceed 2000, so X^2
                    # overflows fp16 without the pre-scale
                    nc.scalar.activation(
                        sq[:],
                        X[:, c * 1024 : (c + 1) * 1024],
                        AF.Square,
                        scale=1.0 / 64,
                    )
                    st = ps_d.tile([P, 1024], F32, name="st", tag="d")
                    st_s = st[:, 0:512]
                    st_q = st[:, 512:1024]
                    for ec in range(EC):
                        nc.tensor.matmul(
                            st_s,
                            ones128[:],
                            X[:, _col(c, ec) : _col(c, ec) + 512],
                            start=(ec == 0),
                            stop=(ec == EC - 1),
                        )
                    for ec in range(EC):
                        nc.tensor.matmul(
                            st_q,
                            ones128[:],
                            sq[:, ec * 512 : (ec + 1) * 512],
                            start=(ec == 0),
                            stop=(ec == EC - 1),
                        )
                    cs = slice(c * 512, (c + 1) * 512)
                    t1 = rpool.tile([P, 512], F32, name="t1", tag="t1")
                    # t1 = (sum/64)^2 = sum^2/4096 (one PSUM input only)
                    nc.scalar.activation(t1[:], st_s, AF.Square, scale=1.0 / 64)
                    # t1 <- E*var/4096 = (sumsq - sum^2/E)/4096
                    nc.vector.scalar_tensor_tensor(
                        t1[:], t1[:], -1.0 / E, st_q, op0=OP.mult, op1=OP.add
                    )
                    nc.scalar.activation(
                        rstd_all[b][:, cs],
                        t1[:],
                        AF.Abs_reciprocal_sqrt,
                        bias=epsc[:],
                        scale=4096.0 / E,
                    )
                    nc.vector.scalar_tensor_tensor(
                        mrstd_all[b][:, cs],
                        st_s,
                        1.0 / E,
                        rstd_all[b][:, cs],
                        op0=OP.mult,
                        op1=OP.mult,
                    )

                def t_apply():
                    cs = slice(c * 512, (c + 1) * 512)
                    up = uppool.tile([P, 1024], F16, name="up", tag="up")
                    tt = uppool.tile([P, 1024], F16, name="tt", tag="tt")
                    # alternate GP/DVE per chunk so neither engine serializes
                    # the apply stage (GP runs ~2.3x slower per element)
                    up_eng = nc.gpsimd if c % 2 == 0 else nc.vector
                    add_eng = nc.gpsimd if c % 2 == 1 else nc.vector
                    for pt in range(EC):
                        up_eng.tensor_scalar(
                            up[:, pt * 512 : (pt + 1) * 512],
                            mrstd_all[b][:, cs],
                            ng_col[:, pt : pt + 1],
                            be_col[:, pt : pt + 1],
                            op0=OP.mult,
                            op1=OP.add,
                        )
                        nc.vector.scalar_tensor_tensor(
                            tt[:, pt * 512 : (pt + 1) * 512],
                            X[:, _col(c, pt) : _col(c, pt) + 512],
                            g_col[:, pt : pt + 1],
                            rstd_all[b][:, cs],
                            op0=OP.mult,
                            op1=OP.mult,
                        )
                    add_eng.tensor_add(
                        X[:, c * 1024 : (c + 1) * 1024], tt[:], up[:]
                    )

                return [t_stats, t_apply]

            def mlp_chunk_task(b, l, c):
                X = Hbuf[b][(l + 1) % 2]

                def t():
                    psa = ps_d.tile([P, 1024], F32, name="psa", tag="d")
                    for mc in range(EC):
                        for ec in range(EC):
                            nc.tensor.matmul(
                                psa[:, mc * 512 : (mc + 1) * 512],
                                w_sb["W1"][l][ec][:, mc * P : (mc + 1) * P],
                                X[:, _col(c, ec) : _col(c, ec) + 512],
                                start=(ec == 0),
                                stop=(ec == EC - 1),
                            )
                    a = apool.tile([P, 1024], F16, name="a", tag="a")
                    for mc in range(EC):
                        nc.scalar.activation(
                            a[:, mc * 512 : (mc + 1) * 512],
                            psa[:, mc * 512 : (mc + 1) * 512],
                            AF.Relu,
                            bias=bm1_sb[l][:, mc : mc + 1],
                        )
                    psm = ps_d.tile([P, 1024], F32, name="psm", tag="d")
                    for oc in range(EC):
                        for mc in range(EC):
                            nc.tensor.matmul(
                                psm[:, oc * 512 : (oc + 1) * 512],
                                w_sb["W2"][l][mc][:, oc * P : (oc + 1) * P],
                                a[:, mc * 512 : (mc + 1) * 512],
                                start=(mc == 0),
                                stop=(mc == EC - 1),
                            )
                    for oc in range(EC):
                        xs_ = slice(_col(c, oc), _col(c, oc) + 512)
                        nc.vector.scalar_tensor_tensor(
                            X[:, xs_],
                            psm[:, oc * 512 : (oc + 1) * 512],
                            bm2_sb[l][:, oc : oc + 1],
                            X[:, xs_],
                            op0=OP.add,
                            op1=OP.add,
                        )

                return [t]

            def ro_chunk_tasks(b, c):
                X = Hbuf[b][L % 2]

                def t():
                    ros = ropool.tile([P, 1024], F16, name="ros", tag="ros")
                    nc.vector.tensor_mul(
                        ros[:],
                        X[:, c * 1024 : (c + 1) * 1024],
                        wout_sb[:, c * 1024 : (c + 1) * 1024],
                    )
                    st = ps_d.tile([P, 1024], F32, name="str", tag="d")
                    for ec in range(EC):
                        nc.tensor.matmul(
                            st[:, 0:512],
                            ones128[:],
                            ros[:, ec * 512 : (ec + 1) * 512],
                            start=(ec == 0),
                            stop=(ec == EC - 1),
                        )
                    nc.vector.reduce_sum(
                        rsum4[b][:, c : c + 1], st[:, 0:512], axis=mybir.AxisListType.X
                    )

                return [t]

            def ro_final_task(b):
                def t():
                    rsc = ropool.tile([P, 1], F32, name="rsc", tag="rsc")
                    nc.vector.reduce_sum(
                        rsc[:], rsum4[b][:], axis=mybir.AxisListType.X
                    )
                    ob = ropool.tile([1, 1], F32, name="ob", tag="ob")
                    nc.scalar.activation(
                        ob[:], rsc[0:1, :], AF.Identity, bias=bout_sb[:]
                    )
                    nc.sync.dma_start(d_out[b : b + 1, :], ob[:])

                return [t]

            def dense_tasks(b, l):
                """Stage-major dense phase for layer l: each stage sweeps all
                chunks so per-chunk serial chains (stats->rowmath->apply)
                pipeline across chunks instead of stalling the next stage."""
                X = Hbuf[b][(l + 1) % 2]
                S = []
                ln1 = [
                    ln_chunk_tasks(b, X, c, g1_sb[l], ng1_sb[l], be1_sb[l])
                    for c in range(CH)
                ]
                ln2 = [
                    ln_chunk_tasks(b, X, c, g2_sb[l], ng2_sb[l], be2_sb[l])
                    for c in range(CH)
                ]
                S += [t[0] for t in ln1]  # stats c0..c3
                S += [t[1] for t in ln1]  # applies c0..c3
                for c in range(CH):
                    S += mlp_chunk_task(b, l, c)
                S += [t[0] for t in ln2]
                S += [t[1] for t in ln2]
                for c in range(CH):
                    if l + 1 < L:
                        S += qv_chunk_tasks(b, l + 1, c)
                    else:
                        S += ro_chunk_tasks(b, c)
                return S

            def attn_tasks(b, l):
                state = {}
                S = []
                for c in range(CH):
                    S += attn_chunk_tasks(b, l, c, state)
                return S

            def qv0_tasks(b):
                S = []
                for c in range(CH):
                    S += qv_chunk_tasks(b, 0, c)
                return S

            # Phase-disjoint pairing: one stream's attention (PE-heavy)
            # always overlaps the other stream's dense stack (element-engine
            # heavy). Attention phases of the two streams never overlap, so
            # the shared s/o PSUM rings hand off only at phase boundaries
            # (all WAR edges point backwards in emission order: no cycles).
            A, Bb = 0, 1
            sched = []
            sched += _merge(proj_tasks(A), proj_tasks(Bb))
            sched += _merge(qv0_tasks(A) + attn_tasks(A, 0), qv0_tasks(Bb))
            sched += _merge(dense_tasks(A, 0), attn_tasks(Bb, 0))
            sched += _merge(attn_tasks(A, 1), dense_tasks(Bb, 0))
            sched += _merge(dense_tasks(A, 1), attn_tasks(Bb, 1))
            sched += _merge(attn_tasks(A, 2), dense_tasks(Bb, 1))
            # Tail: B's last layer as a wavefront — dense stages advance
            # diagonally across chunks (chunk c stage s at wave c+1+s) behind
            # B's own attention chunks, so each wave has PE work and per-chunk
            # serial chains pipeline across the diagonal.
            stateB = {}
            XB = Hbuf[Bb][L % 2]
            stages = []
            for c in range(CH):
                ln1 = ln_chunk_tasks(Bb, XB, c, g1_sb[2], ng1_sb[2], be1_sb[2])
                ln2 = ln_chunk_tasks(Bb, XB, c, g2_sb[2], ng2_sb[2], be2_sb[2])
                stages.append(
                    [
                        [ln1[0]],
                        [ln1[1]],
                        mlp_chunk_task(Bb, 2, c),
                        [ln2[0]],
                        [ln2[1]],
                        ro_chunk_tasks(Bb, c),
                    ]
                )
            NS = 6
            waves = []
            for w in range(CH + NS):
                wv = []
                if w < CH:
                    wv += attn_chunk_tasks(Bb, 2, w, stateB)
                for c in range(CH):
                    s = w - 1 - c
                    if 0 <= s < NS:
                        wv += stages[c][s]
                waves.append(wv)
            tailB = [t for wv in waves for t in wv]
            sched += _merge(dense_tasks(A, 2) + ro_final_task(A), tailB)
            sched += ro_final_task(Bb)
            for t in sched:
                t()

    nc.compile()
    return nc


def _prep_inputs(inputs):
    f = lambda x: np.asarray(x, np.float32)
    xs = f(inputs["xs"])
    xsT = np.ascontiguousarray(xs.transpose(0, 2, 1)).astype(NPF16)  # [B, D, N]
    Wq, Wk = f(inputs["Wq"]), f(inputs["Wk"])
    Wqk = np.stack([Wq[l] @ Wk[l].T for l in range(L)]).astype(NPF16)
    WoutT = f(inputs["Wout"]).reshape(N, E).T  # [E, N]
    woutP = np.ascontiguousarray(
        WoutT.reshape(EC, P, CH, 512).transpose(1, 2, 0, 3).reshape(P, 2 * N)
    ).astype(NPF16)

    def cols(v, per_l):
        v = f(v)
        if per_l:
            return np.ascontiguousarray(v.reshape(L, EC, P).transpose(0, 2, 1))
        return np.ascontiguousarray(v.reshape(EC, P).T)

    groups = [
        cols(inputs["bm1"], True),
        cols(inputs["bm2"], True),
        cols(inputs["be1"], True),
        cols(inputs["be2"], True),
        cols(inputs["g1"], True),
        cols(inputs["g2"], True),
        cols(-f(inputs["g1"]), True),
        cols(-f(inputs["g2"]), True),
    ]
    colpack = np.concatenate(
        [cols(inputs["b_in"], False)]
        + [g.transpose(1, 0, 2).reshape(P, L * EC) for g in groups],
        axis=1,
    )
    common = {
        "Win": f(inputs["Win"]).astype(NPF16),
        "Wqk": Wqk,
        "Wv": f(inputs["Wv"]).astype(NPF16),
        "W1": f(inputs["W1"]).astype(NPF16),
        "W2": f(inputs["W2"]).astype(NPF16),
        "woutP": woutP,
        "colpack": np.ascontiguousarray(colpack),
        "b_out": f(inputs["b_out"]).reshape(1, 1),
    }
    in_maps = []
    for c in range(NCORES):
        m = dict(common)
        m["xsT"] = np.ascontiguousarray(xsT[c * BL : (c + 1) * BL])
        in_maps.append(m)
    return in_maps


def get_program():
    if "nc" not in _CACHE:
        _CACHE["nc"] = _build()
    return _CACHE["nc"]


def kernel(**inputs) -> np.ndarray:
    nc = get_program()
    in_maps = _prep_inputs(inputs)
    res = run_bass_kernel_spmd(nc, in_maps, list(range(NCORES)))
    out = np.concatenate([res.results[c]["out"] for c in range(NCORES)], axis=0)
    return out.astype(np.float32)


# revision 33
# speedup vs baseline: 1.0326x; 1.0029x over previous
"""Trainium2 Bass kernel for nn_EncoderTransformer_61194694033513.

Data-parallel over batch B=16 across 8 NeuronCores (2 batch elems per core).
Per core the forward runs out of SBUF, activations feature-major HT[e, tok]
in fp16, interleaved-chunk column layout: col = c*1024 + ec*512 + t'
(c = tok//512, t' = tok%512, ec = feat//128, partition = feat%128), so every
per-chunk op is one contiguous [P,1024] instruction covering both feature
chunks.

v3 structure:
- Wk folded into Wqk = Wq @ Wk^T on host; scores S = (H Wqk) H^T use live H
  tiles as keys; H double-buffered across layers (attention reads Hcur,
  writes Hcur+O into Hnxt).
- LN stats replicated across partitions via ones[P,128] lhsT matmuls so all
  row math runs 128-lane wide; no partition broadcasts.
- The dense stack (LN1+MLP+LN2+next-layer q'/v) is token-chunk-local, so it
  is embedded chunk-by-chunk inside the same stream's attention: attn(c0),
  attn(c1)||dense(c0), attn(c2)||dense(c1), attn(c3)||dense(c2), dense(c3).
  v is double-buffered across layers (next layer's v tiles are produced
  while this layer's attention still reads the old ones).
- The two batch elems additionally interleave half a layer out of phase.
- PSUM: s [P,512] ring2 (2 banks) + per-stream o [P,1024] ring1 (4 banks)
  + shared dense ring1 [P,1024] (2 banks) = 8 banks.
"""

import sys

import numpy as np

for _p in (
    "/opt/trn_rl_repo",
    "/opt/pypackages",
    "/root/.axon_site",
    "/root/.axon_site/_ro/trn_rl_repo",
    "/root/.axon_site/_ro/pypackages",
):
    if _p not in sys.path:
        sys.path.append(_p)

import concourse.bass as bass  # noqa: E402
import concourse.bacc as bacc  # noqa: E402
import concourse.mybir as mybir  # noqa: E402
from concourse import tile  # noqa: E402
from concourse.bass_utils import run_bass_kernel_spmd  # noqa: E402

B, N, D, E, L = 16, 2048, 128, 256, 3
NCORES = 8
BL = B // NCORES
P = 128
EC = E // P  # 2 feature chunks
CH = N // 512  # 4 token chunks
JT = N // P  # 16 key tiles
EPS = 1e-5
F32 = mybir.dt.float32
F16 = mybir.dt.float16
NPF16 = np.float16
AF = mybir.ActivationFunctionType
OP = mybir.AluOpType

_CACHE = {}


def _col(c, ec, off=0):
    return c * 1024 + ec * 512 + off


def _merge(xs, ys):
    """Proportionally interleave two task lists."""
    n, m = len(xs), len(ys)
    out = []
    i = j = 0
    while i < n or j < m:
        if j >= m or (i < n and i * (m + 1) <= j * (n + 1)):
            out.append(xs[i])
            i += 1
        else:
            out.append(ys[j])
            j += 1
    return out


def _build():
    nc = bacc.Bacc("TRN2", target_bir_lowering=False, debug=False, num_devices=NCORES)

    d_xsT = nc.declare_dram_parameter("xsT", [BL, P, N], F16, isOutput=False)
    d_Win = nc.declare_dram_parameter("Win", [D, E], F16, isOutput=False)
    WNAMES = ("Wqk", "Wv", "W1", "W2")
    d_W = {
        nm: nc.declare_dram_parameter(nm, [L, E, E], F16, isOutput=False)
        for nm in WNAMES
    }
    d_woutP = nc.declare_dram_parameter("woutP", [P, 2 * N], F16, isOutput=False)
    NCOL = 2 + 8 * L * EC
    d_colpack = nc.declare_dram_parameter("colpack", [P, NCOL], F32, isOutput=False)
    d_bout = nc.declare_dram_parameter("b_out", [1, 1], F32, isOutput=False)
    d_out = nc.declare_dram_parameter("out", [BL, 1], F32, isOutput=True)

    with tile.TileContext(nc) as tc:
        from contextlib import ExitStack

        with ExitStack() as ctx:
            cpool = ctx.enter_context(tc.tile_pool(name="const", bufs=1))
            hpool = ctx.enter_context(tc.tile_pool(name="acts", bufs=1))
            sqpool = ctx.enter_context(tc.tile_pool(name="sqp", bufs=2))
            spool = ctx.enter_context(tc.tile_pool(name="srelu", bufs=3))
            apool = ctx.enter_context(tc.tile_pool(name="mlpa", bufs=2))
            rpool = ctx.enter_context(tc.tile_pool(name="rowm", bufs=2))
            uppool = ctx.enter_context(tc.tile_pool(name="applyp", bufs=4))
            ropool = ctx.enter_context(tc.tile_pool(name="ro", bufs=2))

            PS = bass.MemorySpace.PSUM
            # s: 2x[P,1024] (4 banks), o: 1x[P,1024] (2), d: 1x[P,1024] (2)
            ps_s = ctx.enter_context(tc.tile_pool(name="ps_s", bufs=2, space=PS))
            ps_o = ctx.enter_context(tc.tile_pool(name="ps_o", bufs=1, space=PS))
            ps_d = ctx.enter_context(tc.tile_pool(name="ps_d", bufs=1, space=PS))

            xs_tiles = []
            for b in range(BL):
                xt = hpool.tile([P, N], F16, name=f"xst{b}", tag=f"xst{b}")
                for c in range(CH):
                    cs = slice(c * 512, (c + 1) * 512)
                    nc.sync.dma_start(xt[:, cs], d_xsT[b][:, cs])
                xs_tiles.append(xt)

            win_sb = cpool.tile([P, E], F16, name="win", tag="win")
            nc.sync.dma_start(win_sb[:], d_Win[:])
            colpack = cpool.tile([P, NCOL], F32, name="colpack", tag="colpack")
            nc.sync.dma_start(colpack[:], d_colpack[:])
            binp_sb = colpack[:, 0:EC]

            def col_views(base):
                return [
                    colpack[
                        :, 2 + base * L * EC + l * EC : 2 + base * L * EC + (l + 1) * EC
                    ]
                    for l in range(L)
                ]

            bm1_sb = col_views(0)
            bm2_sb = col_views(1)
            be1_sb = col_views(2)
            be2_sb = col_views(3)
            g1_sb = col_views(4)
            g2_sb = col_views(5)
            ng1_sb = col_views(6)
            ng2_sb = col_views(7)

            w_sb = {nm: [None] * L for nm in WNAMES}
            w_big = {}
            for nm in WNAMES:
                w_big[nm] = [
                    cpool.tile([P, L * E], F16, name=f"{nm}B{ec}", tag=f"{nm}B{ec}")
                    for ec in range(EC)
                ]
            for l in range(L):
                for nm in WNAMES:
                    for ec in range(EC):
                        nc.sync.dma_start(
                            w_big[nm][ec][:, l * E : (l + 1) * E],
                            d_W[nm][l, ec * P : (ec + 1) * P, :],
                        )
            for nm in WNAMES:
                for l in range(L):
                    w_sb[nm][l] = [
                        w_big[nm][ec][:, l * E : (l + 1) * E] for ec in range(EC)
                    ]

            wout_sb = cpool.tile([P, 2 * N], F16, name="woutp", tag="woutp")
            nc.sync.dma_start(wout_sb[:], d_woutP[:])
            bout_sb = cpool.tile([1, 1], F32, name="bout", tag="bout")
            nc.sync.dma_start(bout_sb[:], d_bout[:])

            ones128 = cpool.tile([P, P], F16, name="ones128", tag="ones128")
            nc.vector.memset(ones128[:], 1.0)
            epsc = cpool.tile([P, 1], F32, name="epsc", tag="epsc")
            nc.vector.memset(epsc[:], EPS)
            # pre-warm ACT function tables during the initial DMA wait so the
            # first real activation doesn't pay ACT_TABLE_LOAD
            warm = cpool.tile([1, 1], F32, name="warm", tag="warm")
            nc.vector.memset(warm[:], 1.0)
            for fn in (AF.Relu, AF.Square, AF.Abs_reciprocal_sqrt, AF.Identity):
                nc.scalar.activation(warm[:], warm[:], fn)

            Hbuf = [
                [
                    hpool.tile([P, 2 * N], F16, name=f"H{b}{i}", tag=f"H{b}{i}")
                    for i in range(2)
                ]
                for b in range(BL)
            ]
            qT = [
                hpool.tile([P, 2 * N], F16, name=f"qT{b}", tag=f"qT{b}")
                for b in range(BL)
            ]
            vbuf = [
                [
                    hpool.tile([P, JT * E], F16, name=f"v{b}{i}", tag=f"v{b}{i}")
                    for i in range(2)
                ]
                for b in range(BL)
            ]
            rstd_all = [
                hpool.tile([P, N], F16, name=f"rstd{b}", tag=f"rstd{b}")
                for b in range(BL)
            ]
            mrstd_all = [
                hpool.tile([P, N], F16, name=f"mrstd{b}", tag=f"mrstd{b}")
                for b in range(BL)
            ]
            rsum4 = [
                hpool.tile([P, CH], F32, name=f"rs4{b}", tag=f"rs4{b}")
                for b in range(BL)
            ]

            relu_ctr = [0, 0]
            sq_ctr = [0, 0]

            def proj_tasks(b):
                def mk(c):
                    def t():
                        # ps_s ring (idle pre-attention): deeper pipelining
                        # than the shared dense ring during startup
                        ps = ps_s.tile([P, 1024], F32, name="psp", tag="s")
                        for ec in range(EC):
                            nc.tensor.matmul(
                                ps[:, ec * 512 : (ec + 1) * 512],
                                win_sb[:, ec * P : (ec + 1) * P],
                                xs_tiles[b][:, c * 512 : (c + 1) * 512],
                            )
                        for ec in range(EC):
                            nc.scalar.activation(
                                Hbuf[b][0][:, _col(c, ec) : _col(c, ec) + 512],
                                ps[:, ec * 512 : (ec + 1) * 512],
                                AF.Identity,
                                bias=binp_sb[:, ec : ec + 1],
                            )

                    return t

                return [mk(c) for c in range(CH)]

            def qv_chunk_tasks(b, l, c):
                """q' chunk c and v key tiles 4c..4c+3 for layer l (reads
                H_l = Hbuf[b][l%2], writes qT and vbuf[b][l%2])."""
                cur = Hbuf[b][l % 2]
                vd = vbuf[b][l % 2]
                tasks = []

                def tq():
                    ps = ps_d.tile([P, 1024], F32, name="psq", tag="d")
                    for dc in range(EC):
                        for ec in range(EC):
                            nc.tensor.matmul(
                                ps[:, dc * 512 : (dc + 1) * 512],
                                w_sb["Wqk"][l][ec][:, dc * P : (dc + 1) * P],
                                cur[:, _col(c, ec) : _col(c, ec) + 512],
                                start=(ec == 0),
                                stop=(ec == EC - 1),
                            )
                    if c % 2 == 0:
                        nc.scalar.copy(qT[b][:, c * 1024 : (c + 1) * 1024], ps[:])
                    else:
                        nc.vector.tensor_copy(
                            qT[b][:, c * 1024 : (c + 1) * 1024], ps[:]
                        )

                tasks.append(tq)

                def mk_v(t2):
                    def t():
                        ps = ps_d.tile([P, 1024], F32, name="psv", tag="d")
                        for h in range(2):
                            kt = 2 * t2 + h
                            for ec in range(EC):
                                nc.tensor.matmul(
                                    ps[:, h * E : (h + 1) * E],
                                    cur[
                                        :,
                                        _col(kt // 4, ec, (kt % 4) * P) : _col(
                                            kt // 4, ec, (kt % 4) * P
                                        )
                                        + P,
                                    ],
                                    w_sb["Wv"][l][ec][:],
                                    start=(ec == 0),
                                    stop=(ec == EC - 1),
                                )
                        dst = vd[:, 2 * t2 * E : (2 * t2 + 2) * E]
                        # alternate engines: these copies race the next attn
                        # phase's o-matmuls and must not pile on one queue
                        if t2 % 2 == 0:
                            nc.scalar.copy(dst, ps[:, 0 : 2 * E])
                        else:
                            nc.vector.tensor_copy(dst, ps[:, 0 : 2 * E])

                    return t

                tasks.append(mk_v(2 * c))
                tasks.append(mk_v(2 * c + 1))
                return tasks

            def attn_chunk_tasks(b, l, c, state):
                cur = Hbuf[b][l % 2]
                nxt = Hbuf[b][(l + 1) % 2]
                vd = vbuf[b][l % 2]
                tasks = []

                def mk_j2(j2):
                    def t():
                        if j2 == 0:
                            state["o"] = ps_o.tile([P, 1024], F32, name="o", tag="o")
                        o_ps = state["o"]
                        s_ps = ps_s.tile([P, 1024], F32, name="s", tag="s")
                        for h in range(2):
                            j = 2 * j2 + h
                            for dc in range(EC):
                                nc.tensor.matmul(
                                    s_ps[:, h * 512 : (h + 1) * 512],
                                    cur[
                                        :,
                                        _col(j // 4, dc, (j % 4) * P) : _col(
                                            j // 4, dc, (j % 4) * P
                                        )
                                        + P,
                                    ],
                                    qT[b][:, _col(c, dc) : _col(c, dc) + 512],
                                    start=(dc == 0),
                                    stop=(dc == EC - 1),
                                )
                        sr = spool.tile([P, 1024], F16, name="sr", tag="sr")
                        if relu_ctr[b] % 8 < 3:
                            nc.vector.tensor_relu(sr[:], s_ps[:])
                        else:
                            nc.scalar.activation(sr[:], s_ps[:], AF.Relu)
                        relu_ctr[b] += 1
                        for h in range(2):
                            j = 2 * j2 + h
                            for oc in range(EC):
                                nc.tensor.matmul(
                                    o_ps[:, oc * 512 : (oc + 1) * 512],
                                    vd[:, j * E + oc * P : j * E + (oc + 1) * P],
                                    sr[:, h * 512 : (h + 1) * 512],
                                    start=(j == 0),
                                    stop=(j == JT - 1),
                                )

                    return t

                for j2 in range(JT // 2):
                    tasks.append(mk_j2(j2))

                def t_add():
                    nc.vector.tensor_add(
                        nxt[:, c * 1024 : (c + 1) * 1024],
                        cur[:, c * 1024 : (c + 1) * 1024],
                        state["o"][:],
                    )

                tasks.append(t_add)
                return tasks

            def ln_chunk_tasks(b, X, c, g_col, ng_col, be_col):
                def t_stats():
                    sq = sqpool.tile([P, 1024], F16, name="sq", tag="sq")
                    # sq = (X/64)^2: pre-LN |X| can exceed 2000, so X^2
                    # overflows fp16 without the pre-scale
                    nc.scalar.activation(
                        sq[:],
                        X[:, c * 1024 : (c + 1) * 1024],
                        AF.Square,
                        scale=1.0 / 64,
                    )
                    st = ps_d.tile([P, 1024], F32, name="st", tag="d")
                    st_s = st[:, 0:512]
                    st_q = st[:, 512:1024]
                    for ec in range(EC):
                        nc.tensor.matmul(
                            st_s,
                            ones128[:],
                            X[:, _col(c, ec) : _col(c, ec) + 512],
                            start=(ec == 0),
                            stop=(ec == EC - 1),
                        )
                    for ec in range(EC):
                        nc.tensor.matmul(
                            st_q,
                            ones128[:],
                            sq[:, ec * 512 : (ec + 1) * 512],
                            start=(ec == 0),
                            stop=(ec == EC - 1),
                        )
                    cs = slice(c * 512, (c + 1) * 512)
                    t1 = rpool.tile([P, 512], F32, name="t1", tag="t1")
                    # t1 = (sum/64)^2 = sum^2/4096 (one PSUM input only)
                    nc.scalar.activation(t1[:], st_s, AF.Square, scale=1.0 / 64)
                    # t1 <- E*var/4096 = (sumsq - sum^2/E)/4096
                    nc.vector.scalar_tensor_tensor(
                        t1[:], t1[:], -1.0 / E, st_q, op0=OP.mult, op1=OP.add
                    )
                    nc.scalar.activation(
                        rstd_all[b][:, cs],
                        t1[:],
                        AF.Abs_reciprocal_sqrt,
                        bias=epsc[:],
                        scale=4096.0 / E,
                    )
                    nc.vector.scalar_tensor_tensor(
                        mrstd_all[b][:, cs],
                        st_s,
                        1.0 / E,
                        rstd_all[b][:, cs],
                        op0=OP.mult,
                        op1=OP.mult,
                    )

                def t_apply():
                    cs = slice(c * 512, (c + 1) * 512)
                    up = uppool.tile([P, 1024], F16, name="up", tag="up")
                    tt = uppool.tile([P, 1024], F16, name="tt", tag="tt")
                    # alternate GP/DVE per chunk so neither engine serializes
                    # the apply stage (GP runs ~2.3x slower per element)
                    up_eng = nc.gpsimd if c % 2 == 0 else nc.vector
                    add_eng = nc.gpsimd if c % 2 == 1 else nc.vector
                    for pt in range(EC):
                        up_eng.tensor_scalar(
                            up[:, pt * 512 : (pt + 1) * 512],
                            mrstd_all[b][:, cs],
                            ng_col[:, pt : pt + 1],
                            be_col[:, pt : pt + 1],
                            op0=OP.mult,
                            op1=OP.add,
                        )
                        nc.vector.scalar_tensor_tensor(
                            tt[:, pt * 512 : (pt + 1) * 512],
                            X[:, _col(c, pt) : _col(c, pt) + 512],
                            g_col[:, pt : pt + 1],
                            rstd_all[b][:, cs],
                            op0=OP.mult,
                            op1=OP.mult,
                        )
                    add_eng.tensor_add(
                        X[:, c * 1024 : (c + 1) * 1024], tt[:], up[:]
                    )

                return [t_stats, t_apply]

            def mlp_chunk_task(b, l, c):
                X = Hbuf[b][(l + 1) % 2]

                def t():
                    psa = ps_d.tile([P, 1024], F32, name="psa", tag="d")
                    for mc in range(EC):
                        for ec in range(EC):
                            nc.tensor.matmul(
                                psa[:, mc * 512 : (mc + 1) * 512],
                                w_sb["W1"][l][ec][:, mc * P : (mc + 1) * P],
                                X[:, _col(c, ec) : _col(c, ec) + 512],
                                start=(ec == 0),
                                stop=(ec == EC - 1),
                            )
                    a = apool.tile([P, 1024], F16, name="a", tag="a")
                    for mc in range(EC):
                        nc.scalar.activation(
                            a[:, mc * 512 : (mc + 1) * 512],
                            psa[:, mc * 512 : (mc + 1) * 512],
                            AF.Relu,
                            bias=bm1_sb[l][:, mc : mc + 1],
                        )
                    psm = ps_d.tile([P, 1024], F32, name="psm", tag="d")
                    for oc in range(EC):
                        for mc in range(EC):
                            nc.tensor.matmul(
                                psm[:, oc * 512 : (oc + 1) * 512],
                                w_sb["W2"][l][mc][:, oc * P : (oc + 1) * P],
                                a[:, mc * 512 : (mc + 1) * 512],
                                start=(mc == 0),
                                stop=(mc == EC - 1),
                            )
                    for oc in range(EC):
                        xs_ = slice(_col(c, oc), _col(c, oc) + 512)
                        nc.vector.scalar_tensor_tensor(
                            X[:, xs_],
                            psm[:, oc * 512 : (oc + 1) * 512],
                            bm2_sb[l][:, oc : oc + 1],
                            X[:, xs_],
                            op0=OP.add,
                            op1=OP.add,
                        )

                return [t]

            def ro_chunk_tasks(b, c):
                X = Hbuf[b][L % 2]

                def t():
                    ros = ropool.tile([P, 1024], F16, name="ros", tag="ros")
                    nc.vector.tensor_mul(
                        ros[:],
                        X[:, c * 1024 : (c + 1) * 1024],
                        wout_sb[:, c * 1024 : (c + 1) * 1024],
                    )
                    st = ps_d.tile([P, 1024], F32, name="str", tag="d")
                    for ec in range(EC):
                        nc.tensor.matmul(
                            st[:, 0:512],
                            ones128[:],
                            ros[:, ec * 512 : (ec + 1) * 512],
                            start=(ec == 0),
                            stop=(ec == EC - 1),
                        )
                    nc.vector.reduce_sum(
                        rsum4[b][:, c : c + 1], st[:, 0:512], axis=mybir.AxisListType.X
                    )

                return [t]

            def ro_final_task(b):
                def t():
                    rsc = ropool.tile([P, 1], F32, name="rsc", tag="rsc")
                    nc.vector.reduce_sum(
                        rsc[:], rsum4[b][:], axis=mybir.AxisListType.X
                    )
                    ob = ropool.tile([1, 1], F32, name="ob", tag="ob")
                    nc.scalar.activation(
                        ob[:], rsc[0:1, :], AF.Identity, bias=bout_sb[:]
                    )
                    nc.sync.dma_start(d_out[b : b + 1, :], ob[:])

                return [t]

            def dense_tasks(b, l):
                """Stage-major dense phase for layer l: each stage sweeps all
                chunks so per-chunk serial chains (stats->rowmath->apply)
                pipeline across chunks instead of stalling the next stage."""
                X = Hbuf[b][(l + 1) % 2]
                S = []
                ln1 = [
                    ln_chunk_tasks(b, X, c, g1_sb[l], ng1_sb[l], be1_sb[l])
                    for c in range(CH)
                ]
                ln2 = [
                    ln_chunk_tasks(b, X, c, g2_sb[l], ng2_sb[l], be2_sb[l])
                    for c in range(CH)
                ]
                S += [t[0] for t in ln1]  # stats c0..c3
                S += [t[1] for t in ln1]  # applies c0..c3
                for c in range(CH):
                    S += mlp_chunk_task(b, l, c)
                S += [t[0] for t in ln2]
                S += [t[1] for t in ln2]
                for c in range(CH):
                    if l + 1 < L:
                        S += qv_chunk_tasks(b, l + 1, c)
                    else:
                        S += ro_chunk_tasks(b, c)
                return S

            def attn_tasks(b, l):
                state = {}
                S = []
                for c in range(CH):
                    S += attn_chunk_tasks(b, l, c, state)
                return S

            def qv0_tasks(b):
                S = []
                for c in range(CH):
                    S += qv_chunk_tasks(b, 0, c)
                return S

            # Phase-disjoint pairing: one stream's attention (PE-heavy)
            # always overlaps the other stream's dense stack (element-engine
            # heavy). Attention phases of the two streams never overlap, so
            # the shared s/o PSUM rings hand off only at phase boundaries
            # (all WAR edges point backwards in emission order: no cycles).
            A, Bb = 0, 1
            sched = []
            sched += _merge(proj_tasks(A), proj_tasks(Bb))
            sched += _merge(qv0_tasks(A) + attn_tasks(A, 0), qv0_tasks(Bb))
            sched += _merge(dense_tasks(A, 0), attn_tasks(Bb, 0))
            sched += _merge(attn_tasks(A, 1), dense_tasks(Bb, 0))
            sched += _merge(dense_tasks(A, 1), attn_tasks(Bb, 1))
            sched += _merge(attn_tasks(A, 2), dense_tasks(Bb, 1))
            # Tail: B's last layer as a wavefront — dense stages advance
            # diagonally across chunks (chunk c stage s at wave c+1+s) behind
            # B's own attention chunks, so each wave has PE work and per-chunk
            # serial chains pipeline across the diagonal.
            stateB = {}
            XB = Hbuf[Bb][L % 2]
            stages = []
            for c in range(CH):
                ln1 = ln_chunk_tasks(Bb, XB, c, g1_sb[2], ng1_sb[2], be1_sb[2])
                ln2 = ln_chunk_tasks(Bb, XB, c, g2_sb[2], ng2_sb[2], be2_sb[2])
                stages.append(
                    [
                        [ln1[0]],
                        [ln1[1]],
                        mlp_chunk_task(Bb, 2, c),
                        [ln2[0]],
                        [ln2[1]],
                        ro_chunk_tasks(Bb, c),
                    ]
                )
            NS = 6
            waves = []
            for w in range(CH + NS):
                wv = []
                if w < CH:
                    wv += attn_chunk_tasks(Bb, 2, w, stateB)
                for c in range(CH):
                    s = w - 1 - c
                    if 0 <= s < NS:
                        wv += stages[c][s]
                waves.append(wv)
            tailB = [t for wv in waves for t in wv]
            sched += _merge(dense_tasks(A, 2) + ro_final_task(A), tailB)
            sched += ro_final_task(Bb)
            for t in sched:
                t()

    nc.compile()
    return nc


def _prep_inputs(inputs):
    f = lambda x: np.asarray(x, np.float32)
    xs = f(inputs["xs"])
    xsT = np.ascontiguousarray(xs.transpose(0, 2, 1)).astype(NPF16)  # [B, D, N]
    Wq, Wk = f(inputs["Wq"]), f(inputs["Wk"])
    Wqk = np.stack([Wq[l] @ Wk[l].T for l in range(L)]).astype(NPF16)
    WoutT = f(inputs["Wout"]).reshape(N, E).T  # [E, N]
    woutP = np.ascontiguousarray(
        WoutT.reshape(EC, P, CH, 512).transpose(1, 2, 0, 3).reshape(P, 2 * N)
    ).astype(NPF16)

    def cols(v, per_l):
        v = f(v)
        if per_l:
            return np.ascontiguousarray(v.reshape(L, EC, P).transpose(0, 2, 1))
        return np.ascontiguousarray(v.reshape(EC, P).T)

    groups = [
        cols(inputs["bm1"], True),
        cols(inputs["bm2"], True),
        cols(inputs["be1"], True),
        cols(inputs["be2"], True),
        cols(inputs["g1"], True),
        cols(inputs["g2"], True),
        cols(-f(inputs["g1"]), True),
        cols(-f(inputs["g2"]), True),
    ]
    colpack = np.concatenate(
        [cols(inputs["b_in"], False)]
        + [g.transpose(1, 0, 2).reshape(P, L * EC) for g in groups],
        axis=1,
    )
    common = {
        "Win": f(inputs["Win"]).astype(NPF16),
        "Wqk": Wqk,
        "Wv": f(inputs["Wv"]).astype(NPF16),
        "W1": f(inputs["W1"]).astype(NPF16),
        "W2": f(inputs["W2"]).astype(NPF16),
        "woutP": woutP,
        "colpack": np.ascontiguousarray(colpack),
        "b_out": f(inputs["b_out"]).reshape(1, 1),
    }
    in_maps = []
    for c in range(NCORES):
        m = dict(common)
        m["xsT"] = np.ascontiguousarray(xsT[c * BL : (c + 1) * BL])
        in_maps.append(m)
    return in_maps


def get_program():
    if "nc" not in _CACHE:
        _CACHE["nc"] = _build()
    return _CACHE["nc"]


def kernel(**inputs) -> np.ndarray:
    nc = get_program()
    in_maps = _prep_inputs(inputs)
    res = run_bass_kernel_spmd(nc, in_maps, list(range(NCORES)))
    out = np.concatenate([res.results[c]["out"] for c in range(NCORES)], axis=0)
    return out.astype(np.float32)
